# revision 1
# baseline (speedup 1.0000x reference)
"""DeepSeek MLA prefill (absorbed) on 8 Trainium2 NeuronCores.

Sharding: tensor-parallel over heads (2 of 16 heads per core) for the
Q-uncompression/attention/O path; the QKV-compression GEMM is sharded over
the hidden (contraction) dim with an AllReduce of the compressed Q; the
O-projection is sharded over output columns after per-head AllGathers of o2.

Attention avoids on-device gather/scatter entirely: scores are computed
dense against the whole kv cache, and the top-k selection (with duplicate
multiplicity) is folded in as a per-(token, position) count matrix:
  softmax over gathered scores == count-weighted dense softmax.
The whole device pipeline runs feature-major (transposed), so every matmul
contracts over the partition dim with 512 moving columns (fp32r full rate).
Cross-core traffic (q_c AllReduce, o2 AllGathers) moves in bf16.
"""

import os
import sys

sys.path.insert(0, "/opt/trn_rl_repo")

import ml_dtypes
import numpy as np

import concourse.bass as bass
import concourse.tile as tile
from concourse import bacc, mybir
from concourse.bass_utils import run_bass_kernel_spmd

F32 = mybir.dt.float32
F32R = mybir.dt.float32r
F16 = mybir.dt.float16
_DTS = {"f16": F16, "f32r": F32R, "f32": F32}
_NPS = {"f16": np.float16, "f32r": np.float32, "f32": np.float32}
_MQ = os.environ.get("KERNEL_DTQ", "f32r")   # stage1 + q_abs + scores path
_MC = os.environ.get("KERNEL_DTC", "f16")    # q_c AllReduce + stage2 path
_MV = os.environ.get("KERNEL_DTV", "f16")    # value + o2 + O-proj path
DT_Q, NP_Q = _DTS[_MQ], _NPS[_MQ]
DT_C, NP_C = _DTS[_MC], _NPS[_MC]
DT_V, NP_V = _DTS[_MV], _NPS[_MV]

N_CORES = 8
M = 512
HID = 7168
HID_C = HID // N_CORES
D_Q_C = 1536
H_LOC = 2
D_ATT = 576
S_KV = 4096
D_KV_C = 512
OUT_C = HID // N_CORES
SM_SCALE = 1.0 / float(np.sqrt(np.float32(D_ATT)))

KH = HID_C // 128    # 7
PQ = D_Q_C // 128    # 12
NSC = S_KV // 128    # 32
DCH = [128, 128, 128, 128, 64]
N_WARM = 20


def build_program():
    nc = bacc.Bacc("TRN2", target_bir_lowering=False, debug=False,
                   num_devices=N_CORES)

    xT = nc.dram_tensor("xT", [HID_C, M], DT_Q, kind="ExternalInput")
    wq = nc.dram_tensor("wq", [HID_C, D_Q_C], DT_Q, kind="ExternalInput")
    wuq = nc.dram_tensor("wuq", [D_Q_C, H_LOC * 192], DT_C,
                         kind="ExternalInput")
    wqk = nc.dram_tensor("wqk", [H_LOC, 128, 512], DT_Q,
                         kind="ExternalInput")
    kvT = nc.dram_tensor("kvT", [D_ATT, S_KV], DT_Q, kind="ExternalInput")
    vv = nc.dram_tensor("vv", [S_KV, D_KV_C], DT_V, kind="ExternalInput")
    cnt = nc.dram_tensor("cnt", [S_KV, M], F32, kind="ExternalInput")
    wo1 = nc.dram_tensor("wo1", [H_LOC, 512, 128], DT_V,
                         kind="ExternalInput")
    wop = nc.dram_tensor("wop", [H_LOC * 128 * N_CORES, OUT_C], DT_V,
                         kind="ExternalInput")
    outT = nc.dram_tensor("outT", [OUT_C, M], F32, kind="ExternalOutput")

    rg = [list(range(N_CORES))]

    with tile.TileContext(nc) as tc, \
            nc.allow_low_precision(reason="fp32r/bf16 matmul pipeline"):
        with tc.tile_pool(name="dram", bufs=1, space="DRAM") as dram:
            qc_loc = dram.tile([D_Q_C, M], DT_C)
            qc_all = dram.tile([D_Q_C, M], DT_C, addr_space="Shared")
            o2_loc = [dram.tile([128, M], DT_V, name=f"o2loc{h}")
                      for h in range(H_LOC)]
            o2_all = [dram.tile([128 * N_CORES, M], DT_V, name=f"o2all{h}",
                                addr_space="Shared") for h in range(H_LOC)]

            # ---------------- PE warmup + stage 1 ------------------------
            with (
                tc.tile_pool(name="s1", bufs=1) as s1,
                tc.tile_pool(name="ps1", bufs=3, space="PSUM") as ps1,
                tc.tile_pool(name="s1o", bufs=3) as s1o,
            ):
                warm = s1.tile([128, 64], F32, name="warm")
                nc.vector.memset(warm[:], 0.0)
                wps = ps1.tile([1, 64], F32, name="wps", tag="wps")
                for i in range(N_WARM):
                    nc.tensor.matmul(wps[:], warm[:, 0:1], warm[:],
                                     start=(i == 0), stop=(i == N_WARM - 1),
                                     skip_group_check=True)
                xt = []
                for k in range(KH):
                    xk = s1.tile([128, M], DT_Q, name=f"x{k}")
                    nc.sync.dma_start(xk[:], xT[k * 128:(k + 1) * 128, :])
                    xt.append(xk)
                # wq in two column groups so p-chunk 0 matmuls start early
                wt = [[None, None] for _ in range(KH)]
                for g in range(2):
                    for k in range(KH):
                        wk = s1.tile([128, D_Q_C // 2], DT_Q,
                                     name=f"w{k}_{g}")
                        nc.sync.dma_start(
                            wk[:], wq[k * 128:(k + 1) * 128,
                                      g * 768:(g + 1) * 768])
                        wt[k][g] = wk
                for p in range(PQ):
                    g, po = divmod(p, 6)
                    acc = ps1.tile([128, M], F32, name="acc1", tag="acc1")
                    for k in range(KH):
                        nc.tensor.matmul(
                            acc[:], wt[k][g][:, po * 128:(po + 1) * 128],
                            xt[k][:], start=(k == 0), stop=(k == KH - 1))
                    ob = s1o.tile([128, M], DT_C, name="qcout", tag="qcout")
                    nc.vector.tensor_copy(ob[:], acc[:])
                    nc.sync.dma_start(qc_loc[p * 128:(p + 1) * 128, :], ob[:])
                nc.gpsimd.collective_compute(
                    "AllReduce", mybir.AluOpType.add, replica_groups=rg,
                    ins=[qc_loc.opt()], outs=[qc_all.opt()])

            # ---------------- stage 2+3: qT, q_absT, q_fullT -------------
            s23_cm = tc.tile_pool(name="s23", bufs=1)
            s23 = s23_cm.__enter__()
            qf = [[None] * 5 for _ in range(H_LOC)]
            with (
                tc.tile_pool(name="s2w", bufs=1) as s2w,
                tc.tile_pool(name="ps2", bufs=2, space="PSUM") as ps2,
            ):
                qct = []
                for k in range(PQ):
                    qk = s2w.tile([128, M], DT_C, name=f"qc{k}")
                    nc.sync.dma_start(qk[:], qc_all[k * 128:(k + 1) * 128, :])
                    qct.append(qk)
                wuqt = []
                for k in range(PQ):
                    wk = s2w.tile([128, H_LOC * 192], DT_C, name=f"wu{k}")
                    nc.sync.dma_start(wk[:], wuq[k * 128:(k + 1) * 128, :])
                    wuqt.append(wk)
                wqkt = []
                for h in range(H_LOC):
                    wh = s2w.tile([128, 512], DT_Q, name=f"wqk{h}")
                    nc.sync.dma_start(wh[:], wqk[h])
                    wqkt.append(wh)
                nope = []
                for h in range(H_LOC):
                    off = h * 192
                    acc = ps2.tile([128, M], F32, name="acc2", tag="acc2")
                    for k in range(PQ):
                        nc.tensor.matmul(
                            acc[:], wuqt[k][:, off:off + 128], qct[k][:],
                            start=(k == 0), stop=(k == PQ - 1))
                    nb = s23.tile([128, M], DT_Q, name=f"nope{h}")
                    nc.vector.tensor_copy(nb[:], acc[:])
                    nope.append(nb)
                    off = h * 192 + 128
                    acc = ps2.tile([64, M], F32, name="accp", tag="accp")
                    for k in range(PQ):
                        nc.tensor.matmul(
                            acc[:], wuqt[k][:, off:off + 64], qct[k][:],
                            start=(k == 0), stop=(k == PQ - 1))
                    pb = s23.tile([64, M], DT_Q, name=f"pe{h}")
                    nc.vector.tensor_copy(pb[:], acc[:])
                    qf[h][4] = pb
                for h in range(H_LOC):
                    for c in range(4):
                        acc = ps2.tile([128, M], F32, name="acc3", tag="acc3")
                        nc.tensor.matmul(
                            acc[:], wqkt[h][:, c * 128:(c + 1) * 128],
                            nope[h][:], start=True, stop=True)
                        qb = s23.tile([128, M], DT_Q, name=f"qf{h}_{c}")
                        nc.vector.tensor_copy(qb[:], acc[:])
                        qf[h][c] = qb

            # ---------------- attention ---------------------------------
            att_cm = tc.tile_pool(name="att", bufs=1)
            att = att_cm.__enter__()
            ones_col_f = att.tile([128, 1], F32, name="ones_col_f")
            nc.vector.memset(ones_col_f[:], 1.0)
            ones_col = att.tile([128, 1], DT_V, name="ones_col")
            nc.vector.tensor_copy(ones_col[:], ones_col_f[:])
            ones_row_f = att.tile([1, 128], F32, name="ones_row_f")
            nc.vector.memset(ones_row_f[:], 1.0)
            ones_row = att.tile([1, 128], DT_V, name="ones_row")
            nc.vector.tensor_copy(ones_row[:], ones_row_f[:])
            pt = [att.tile([128, NSC * M], DT_V, name=f"pt{h}")
                  for h in range(H_LOC)]
            z_sb = [att.tile([1, M], F32, name=f"z{h}") for h in range(H_LOC)]
            rz = [att.tile([1, M], DT_V, name=f"rz{h}")
                  for h in range(H_LOC)]
            zb_sb = [att.tile([128, M], F32, name=f"zs{h}")
                     for h in range(H_LOC)]

            with (
                tc.tile_pool(name="kvs", bufs=4) as kvs,
                tc.tile_pool(name="cnts", bufs=4) as cnts,
                tc.tile_pool(name="exps", bufs=4) as exps,
                tc.tile_pool(name="psS", bufs=4, space="PSUM") as psS,
                tc.tile_pool(name="psZ", bufs=1, space="PSUM") as psZ,
            ):
                z_ps = [psZ.tile([1, M], F32, name=f"zp{h}")
                        for h in range(H_LOC)]
                for sc in range(NSC):
                    kvc = []
                    d0 = 0
                    for j, dch in enumerate(DCH):
                        kj = kvs.tile([dch, 128], DT_Q, name=f"kv{j}",
                                      tag=f"kv{j}")
                        nc.sync.dma_start(
                            kj[:], kvT[d0:d0 + dch, sc * 128:(sc + 1) * 128])
                        kvc.append(kj)
                        d0 += dch
                    cc = cnts.tile([128, M], F32, name="cc", tag="cc")
                    nc.sync.dma_start(cc[:], cnt[sc * 128:(sc + 1) * 128, :])
                    for h in range(H_LOC):
                        acc = psS.tile([128, M], F32, name="accS", tag="accS")
                        for j in range(5):
                            nc.tensor.matmul(
                                acc[:], kvc[j][:], qf[h][j][:],
                                start=(j == 0), stop=(j == 4))
                        ex = exps.tile([128, M], F32, name="ex", tag="ex")
                        nc.scalar.activation(
                            ex[:], acc[:], mybir.ActivationFunctionType.Exp,
                            scale=SM_SCALE)
                        psl = pt[h][:, sc * M:(sc + 1) * M]
                        nc.vector.tensor_mul(psl, ex[:], cc[:])
                        nc.tensor.matmul(
                            z_ps[h][:], ones_col[:], psl,
                            start=(sc == 0), stop=(sc == NSC - 1),
                            skip_group_check=True)
                for h in range(H_LOC):
                    nc.vector.tensor_copy(z_sb[h][:], z_ps[h][:])
                    nc.vector.reciprocal(rz[h][:], z_sb[h][:])

            # 1/Z broadcast rows (PE) before the value phase claims PSUM
            with tc.tile_pool(name="psB", bufs=2, space="PSUM") as psB:
                for h in range(H_LOC):
                    zb = psB.tile([128, M], F32, name="zb", tag="zb")
                    nc.tensor.matmul(zb[:], ones_row[:], rz[h][:],
                                     start=True, stop=True)
                    nc.vector.tensor_copy(zb_sb[h][:], zb[:])

            # value matmuls + O-bmm + AllGather, head-sequential so the
            # first head's AllGather overlaps the second head's matmuls
            o_sb = [att.tile([128, M], DT_V, name=f"o_{c}")
                    for c in range(4)]
            with (
                tc.tile_pool(name="vs", bufs=4) as vs,
                tc.tile_pool(name="psO", bufs=1, space="PSUM") as psO,
                tc.tile_pool(name="ps5", bufs=2, space="PSUM") as ps5,
                tc.tile_pool(name="s5", bufs=1) as s5,
            ):
                wo1t = [[None] * 4 for _ in range(H_LOC)]
                for h in range(H_LOC):
                    for kc in range(4):
                        wk = s5.tile([128, 128], DT_V, name=f"wo1_{h}_{kc}")
                        nc.sync.dma_start(
                            wk[:], wo1[h][kc * 128:(kc + 1) * 128, :])
                        wo1t[h][kc] = wk
                o_ps = [psO.tile([128, M], F32, name=f"op{c}")
                        for c in range(4)]
                for h in range(H_LOC):
                    for sc in range(NSC):
                        vt = vs.tile([128, D_KV_C], DT_V, name="vt",
                                     tag="vt")
                        nc.sync.dma_start(
                            vt[:], vv[sc * 128:(sc + 1) * 128, :])
                        psl = pt[h][:, sc * M:(sc + 1) * M]
                        for c in range(4):
                            nc.tensor.matmul(
                                o_ps[c][:],
                                vt[:, c * 128:(c + 1) * 128], psl,
                                start=(sc == 0), stop=(sc == NSC - 1),
                                skip_group_check=True)
                    for c in range(4):
                        nc.vector.tensor_copy(o_sb[c][:], o_ps[c][:])
                    acc = ps5.tile([128, M], F32, name="acc5", tag="acc5")
                    for kc in range(4):
                        nc.tensor.matmul(acc[:], wo1t[h][kc][:], o_sb[kc][:],
                                         start=(kc == 0), stop=(kc == 3))
                    o2s = s5.tile([128, M], DT_V, name=f"o2s{h}")
                    nc.vector.tensor_mul(o2s[:], acc[:], zb_sb[h][:])
                    nc.sync.dma_start(o2_loc[h][:], o2s[:])
                    nc.gpsimd.collective_compute(
                        "AllGather", mybir.AluOpType.bypass,
                        replica_groups=rg,
                        ins=[o2_loc[h].opt()], outs=[o2_all[h].opt()])
            att_cm.__exit__(None, None, None)
            s23_cm.__exit__(None, None, None)

            # ---------------- stage 6: O-projection ----------------------
            with (
                tc.tile_pool(name="s6", bufs=1) as s6,
                tc.tile_pool(name="ps6", bufs=3, space="PSUM") as ps6,
                tc.tile_pool(name="s6o", bufs=3) as s6o,
            ):
                wopt = []
                for k in range(16):
                    wk = s6.tile([128, OUT_C], DT_V, name=f"wop{k}")
                    nc.sync.dma_start(wk[:], wop[k * 128:(k + 1) * 128, :])
                    wopt.append(wk)
                o2t = []
                for h in range(H_LOC):
                    for k in range(8):
                        ok = s6.tile([128, M], DT_V, name=f"o2a{h}_{k}")
                        nc.sync.dma_start(
                            ok[:], o2_all[h][k * 128:(k + 1) * 128, :])
                        o2t.append((h, k, ok))
                for p in range(OUT_C // 128):
                    acc = ps6.tile([128, M], F32, name="acc6", tag="acc6")
                    for n, (h, k, ok) in enumerate(o2t):
                        # o2_all[h] rank-major rows: global hv row of
                        # (h, rank k_rank, local v) = rank*256 + h*128 + v
                        kk = k * H_LOC + h
                        nc.tensor.matmul(
                            acc[:], wopt[kk][:, p * 128:(p + 1) * 128],
                            ok[:], start=(n == 0), stop=(n == 15))
                    ob = s6o.tile([128, M], F32, name="outb", tag="outb")
                    nc.vector.tensor_copy(ob[:], acc[:])
                    nc.sync.dma_start(outT[p * 128:(p + 1) * 128, :], ob[:])

    nc.compile()
    return nc


def prep_inputs(x, W_cqkv, W_uq, W_qk, kv_cache, W_o1, W_oproj, indices):
    x = np.ascontiguousarray(np.asarray(x, np.float32))
    W_cqkv = np.asarray(W_cqkv, np.float32)
    W_uq = np.asarray(W_uq, np.float32)
    W_qk = np.asarray(W_qk, np.float32)
    kv_cache = np.asarray(kv_cache, np.float32)
    W_o1 = np.asarray(W_o1, np.float32)
    W_oproj = np.asarray(W_oproj, np.float32)
    indices = np.asarray(indices)

    xTf = np.ascontiguousarray(x.T)
    wq_full = W_cqkv[:, D_KV_C:D_KV_C + D_Q_C]
    kvTf = np.ascontiguousarray(kv_cache.T)
    vvf = np.ascontiguousarray(kv_cache[:, :D_KV_C])
    cm = np.zeros((M, S_KV), np.float32)
    np.add.at(cm, (np.arange(M)[:, None], indices), 1.0)
    cntT = np.ascontiguousarray(cm.T)

    in_maps = []
    for i in range(N_CORES):
        r0 = i * HID_C
        h0 = i * H_LOC
        c0 = i * OUT_C
        in_maps.append({
            "xT": xTf[r0:r0 + HID_C].astype(NP_Q),
            "wq": wq_full[r0:r0 + HID_C].astype(NP_Q),
            "wuq": W_uq[:, h0 * 192:(h0 + H_LOC) * 192].astype(NP_C),
            "wqk": W_qk[h0:h0 + H_LOC].astype(NP_Q),
            "kvT": kvTf.astype(NP_Q),
            "vv": vvf.astype(NP_V),
            "cnt": cntT,
            "wo1": W_o1[h0:h0 + H_LOC].astype(NP_V),
            "wop": W_oproj[:, c0:c0 + OUT_C].astype(NP_V),
        })
    return in_maps


_prog_cache = {}


def kernel(x, W_cqkv, W_uq, W_qk, kv_cache, W_o1, W_oproj, indices):
    if "nc" not in _prog_cache:
        _prog_cache["nc"] = build_program()
    nc = _prog_cache["nc"]
    in_maps = prep_inputs(x, W_cqkv, W_uq, W_qk, kv_cache, W_o1, W_oproj,
                          indices)
    trace = bool(int(os.environ.get("KERNEL_TRACE", "0")))
    res = run_bass_kernel_spmd(nc, in_maps, list(range(N_CORES)),
                               trace=trace)
    _prog_cache["last_result"] = res
    out = np.empty((M, HID), np.float32)
    for i in range(N_CORES):
        out[:, i * OUT_C:(i + 1) * OUT_C] = res.results[i]["outT"].T
    return out



# revision 18
# speedup vs baseline: 1.0818x; 1.0818x over previous
"""DeepSeek MLA prefill (absorbed) on 8 Trainium2 NeuronCores.

v2: collective-free front end + per-head pipelined attention.

- Host folds W_uq through W_cqkv (W_eff = Wq @ W_uq), so q for the local
  2 heads is one local GEMM against replicated x — the q_c AllReduce is
  gone entirely. First collective (o2 AllGather, head 0) fires mid-kernel,
  fully hiding the CC entry barrier.
- Attention is head-sequential: head 0 scores->value->o2->AllGather, then
  head 1 likewise; the O-projection accumulates head-0 k-tiles while head
  1's AllGather is in flight (k-outer loop, 7 PSUM banks).
- Whole pipeline fp16 (PSUM f32): halves DMA/SBUF vs f32r; kvT stays
  SBUF-resident across both heads' score passes; cnt/vv stream per chunk.
- Top-k selection is folded in as a count matrix: softmax over gathered
  scores == count-weighted dense softmax against the full kv cache.
"""

import os
import sys

sys.path.insert(0, "/opt/trn_rl_repo")

import numpy as np

import concourse.bass as bass
import concourse.tile as tile
from concourse import bacc, mybir
from concourse.bass_utils import run_bass_kernel_spmd

F32 = mybir.dt.float32
F16 = mybir.dt.float16
NP16 = np.float16

N_CORES = 8
M = 512
HID = 7168
KH = HID // 128          # 56 k-tiles for the fused q GEMM
QL = 384                 # 2 local heads x (128 nope + 64 pe)
H_LOC = 2
D_ATT = 576
S_KV = 4096
NSC = S_KV // 128        # 32 key chunks
D_KV_C = 512
OUT_C = HID // N_CORES   # 896
SM_SCALE = 1.0 / float(np.sqrt(np.float32(D_ATT)))
DCH = [128, 128, 128, 128, 64]
N_WARM = 20


def build_program():
    nc = bacc.Bacc("TRN2", target_bir_lowering=False, debug=False,
                   num_devices=N_CORES)

    xT = nc.dram_tensor("xT", [HID, M], F16, kind="ExternalInput")
    wef = nc.dram_tensor("wef", [HID, QL], F16, kind="ExternalInput")
    wqk = nc.dram_tensor("wqk", [H_LOC, 128, 512], F16, kind="ExternalInput")
    # rows 512:576 = pe dims, rows 576:640 = the same pe dims duplicated so
    # each local head's pe matmul sees matching base partitions
    kvT = nc.dram_tensor("kvT", [640, S_KV], F16, kind="ExternalInput")
    vv = nc.dram_tensor("vv", [S_KV, D_KV_C], F16, kind="ExternalInput")
    cnt = nc.dram_tensor("cnt", [S_KV, M], F16, kind="ExternalInput")
    wo1 = nc.dram_tensor("wo1", [H_LOC, 512, 128], F16, kind="ExternalInput")
    wop = nc.dram_tensor("wop", [H_LOC * 8 * 128, OUT_C], F16,
                         kind="ExternalInput")
    outT = nc.dram_tensor("outT", [OUT_C, M], F16, kind="ExternalOutput")

    rg = [list(range(N_CORES))]

    with tile.TileContext(nc) as tc, \
            nc.allow_low_precision(reason="fp16 matmul pipeline"):
        with tc.tile_pool(name="dram", bufs=1, space="DRAM") as dram:
            o2_loc = [dram.tile([128, M], F16, name=f"o2loc{h}")
                      for h in range(H_LOC)]
            o2_all = [dram.tile([128 * N_CORES, M], F16, name=f"o2all{h}",
                                addr_space="Shared") for h in range(H_LOC)]

            # persistent SBUF tiles
            per_cm = tc.tile_pool(name="per", bufs=1)
            per = per_cm.__enter__()
            kvd = [per.tile([128, S_KV], F16, name=f"kvd{j}")
                   for j in range(5)]
            wqkt = []
            for h in range(H_LOC):
                wh = per.tile([128, 512], F16, name=f"wqk{h}")
                nc.sync.dma_start(wh[:], wqk[h])
                wqkt.append(wh)
            wo1t = [[None] * 4 for _ in range(H_LOC)]
            for h in range(H_LOC):
                for kc in range(4):
                    wk = per.tile([128, 128], F16, name=f"wo1_{h}_{kc}")
                    nc.sync.dma_start(wk[:], wo1[h][kc * 128:(kc + 1) * 128, :])
                    wo1t[h][kc] = wk
            pt = per.tile([128, NSC * M], F16, name="pt")
            qa = [[None] * 5 for _ in range(H_LOC)]
            ones_col_f = per.tile([128, 1], F32, name="ones_col_f")
            nc.vector.memset(ones_col_f[:], 1.0)
            ones_col = per.tile([128, 1], F16, name="ones_col")
            nc.vector.tensor_copy(ones_col[:], ones_col_f[:])
            ones_row_f = per.tile([1, 128], F32, name="ones_row_f")
            nc.vector.memset(ones_row_f[:], 1.0)
            ones_row = per.tile([1, 128], F16, name="ones_row")
            nc.vector.tensor_copy(ones_row[:], ones_row_f[:])
            z_sb = [per.tile([1, M], F32, name=f"z{h}") for h in range(H_LOC)]
            rz = [per.tile([1, M], F16, name=f"rz{h}") for h in range(H_LOC)]
            zb_sb = [per.tile([128, M], F32, name=f"zs{h}")
                     for h in range(H_LOC)]
            o_sb = [per.tile([128, M], F16, name=f"o_{c}") for c in range(4)]
            wopt = [per.tile([128, OUT_C], F16, name=f"wop{n}")
                    for n in range(16)]

            # ---------------- fused q GEMM (stage 1+2) --------------------
            qch = []
            with (
                tc.tile_pool(name="s12w", bufs=1) as s12w,
                tc.tile_pool(name="s12x", bufs=6) as s12x,
                tc.tile_pool(name="s12e", bufs=6) as s12e,
                tc.tile_pool(name="ps12", bufs=1, space="PSUM") as ps12,
            ):
                warm = s12w.tile([128, 64], F32, name="warm")
                nc.vector.memset(warm[:], 0.0)
                wps = ps12.tile([1, 64], F32, name="wps", tag="wps")
                for i in range(N_WARM):
                    nc.tensor.matmul(wps[:], warm[:, 0:1], warm[:],
                                     start=(i == 0), stop=(i == N_WARM - 1),
                                     skip_group_check=True)
                acc12 = [ps12.tile([128, M], F32, name=f"a12_{p}",
                                   tag=f"a12_{p}") for p in range(3)]
                for k in range(KH):
                    xk = s12x.tile([128, M], F16, name="xk", tag="xk")
                    nc.sync.dma_start(xk[:], xT[k * 128:(k + 1) * 128, :])
                    ek = s12e.tile([128, QL], F16, name="ek", tag="ek")
                    nc.sync.dma_start(ek[:], wef[k * 128:(k + 1) * 128, :])
                    for p in range(3):
                        nc.tensor.matmul(
                            acc12[p][:], ek[:, p * 128:(p + 1) * 128], xk[:],
                            start=(k == 0), stop=(k == KH - 1))
                for p in range(3):
                    qc = per.tile([128, M], F16, name=f"qch{p}")
                    nc.vector.tensor_copy(qc[:], acc12[p][:])
                    qch.append(qc)
            for h in range(H_LOC):
                qa[h][4] = qch[2][h * 64:(h + 1) * 64, :]

            # ---------------- q absorb (stage 3) --------------------------
            with tc.tile_pool(name="ps3", bufs=2, space="PSUM") as ps3:
                for h in range(H_LOC):
                    for c in range(4):
                        acc = ps3.tile([128, M], F32, name="acc3", tag="acc3")
                        nc.tensor.matmul(
                            acc[:], wqkt[h][:, c * 128:(c + 1) * 128],
                            qch[h][:], start=True, stop=True)
                        qb = per.tile([128, M], F16, name=f"qa{h}_{c}")
                        nc.vector.tensor_copy(qb[:], acc[:])
                        qa[h][c] = qb

            # ---------------- attention + O path, head-sequential ---------
            with (
                tc.tile_pool(name="cnts", bufs=4) as cnts,
                tc.tile_pool(name="exps", bufs=4) as exps,
                tc.tile_pool(name="vs", bufs=4) as vs,
                tc.tile_pool(name="psS", bufs=2, space="PSUM") as psS,
                tc.tile_pool(name="psZ", bufs=1, space="PSUM") as psZ,
                tc.tile_pool(name="psO", bufs=1, space="PSUM") as psO,
                tc.tile_pool(name="psM", bufs=1, space="PSUM") as psM,
            ):
                for h in range(H_LOC):
                    z_ps = psZ.tile([1, M], F32, name="zp", tag="zp")
                    for sc in range(NSC):
                        if h == 0:
                            # stream kv chunk into the resident tiles;
                            # head 1 reuses them from SBUF
                            for j in range(5):
                                nc.sync.dma_start(
                                    kvd[j][:, sc * 128:(sc + 1) * 128],
                                    kvT[j * 128:(j + 1) * 128,
                                        sc * 128:(sc + 1) * 128])
                        cc = cnts.tile([128, M], F16, name="cc", tag="cc")
                        nc.sync.dma_start(cc[:],
                                          cnt[sc * 128:(sc + 1) * 128, :])
                        acc = psS.tile([128, M], F32, name="accS", tag="accS")
                        for j in range(5):
                            if j < 4:
                                lhsT = kvd[j][:, sc * 128:(sc + 1) * 128]
                                rhs = qa[h][j][:]
                            else:
                                lhsT = kvd[4][h * 64:(h + 1) * 64,
                                              sc * 128:(sc + 1) * 128]
                                rhs = qa[h][4]
                            nc.tensor.matmul(
                                acc[:], lhsT, rhs,
                                start=(j == 0), stop=(j == 4))
                        ex = exps.tile([128, M], F16, name="ex", tag="ex")
                        nc.scalar.activation(
                            ex[:], acc[:], mybir.ActivationFunctionType.Exp,
                            scale=SM_SCALE)
                        psl = pt[:, sc * M:(sc + 1) * M]
                        nc.vector.tensor_mul(psl, ex[:], cc[:])
                        # z matmul pipelined one chunk behind so the PE
                        # never waits on the ACT/DVE chain
                        if sc > 0:
                            pprev = pt[:, (sc - 1) * M:sc * M]
                            nc.tensor.matmul(
                                z_ps[:], ones_col[:], pprev,
                                start=(sc == 1), stop=False,
                                skip_group_check=True)
                    nc.tensor.matmul(
                        z_ps[:], ones_col[:], pt[:, (NSC - 1) * M:NSC * M],
                        start=False, stop=True, skip_group_check=True)
                    nc.vector.tensor_copy(z_sb[h][:], z_ps[:])
                    nc.vector.reciprocal(rz[h][:], z_sb[h][:])

                    o_ps = [psO.tile([128, M], F32, name=f"op{c}",
                                     tag=f"op{c}") for c in range(4)]
                    for sc in range(NSC):
                        vt = vs.tile([128, D_KV_C], F16, name="vt", tag="vt")
                        nc.sync.dma_start(vt[:],
                                          vv[sc * 128:(sc + 1) * 128, :])
                        psl = pt[:, sc * M:(sc + 1) * M]
                        for c in range(4):
                            nc.tensor.matmul(
                                o_ps[c][:], vt[:, c * 128:(c + 1) * 128],
                                psl, start=(sc == 0), stop=(sc == NSC - 1),
                                skip_group_check=True)
                    zb = psM.tile([128, M], F32, name="zb", tag="mm5")
                    nc.tensor.matmul(zb[:], ones_row[:], rz[h][:],
                                     start=True, stop=True)
                    for c in range(4):
                        nc.vector.tensor_copy(o_sb[c][:], o_ps[c][:])
                    nc.vector.tensor_copy(zb_sb[h][:], zb[:])
                    acc5 = psM.tile([128, M], F32, name="acc5", tag="mm5")
                    for kc in range(4):
                        nc.tensor.matmul(acc5[:], wo1t[h][kc][:], o_sb[kc][:],
                                         start=(kc == 0), stop=(kc == 3))
                    o2s = per.tile([128, M], F16, name=f"o2s{h}")
                    nc.vector.tensor_mul(o2s[:], acc5[:], zb_sb[h][:])
                    nc.sync.dma_start(o2_loc[h][:], o2s[:])
                    nc.gpsimd.collective_compute(
                        "AllGather", mybir.AluOpType.bypass,
                        replica_groups=rg,
                        ins=[o2_loc[h].opt()], outs=[o2_all[h].opt()])
                    if h == 0:
                        # O-proj weights, queued behind head-0 traffic so
                        # they don't delay the critical-path streams
                        for n in range(16):
                            nc.sync.dma_start(wopt[n][:],
                                              wop[n * 128:(n + 1) * 128, :])

            # ---------------- O projection (k-outer: h0 then h1) ----------
            with (
                tc.tile_pool(name="s6", bufs=1) as s6,
                tc.tile_pool(name="ps6", bufs=1, space="PSUM") as ps6,
                tc.tile_pool(name="s6o", bufs=3) as s6o,
            ):
                o2t = []
                for h in range(H_LOC):
                    for k in range(8):
                        ok = s6.tile([128, M], F16, name=f"o2a{h}_{k}")
                        nc.sync.dma_start(
                            ok[:], o2_all[h][k * 128:(k + 1) * 128, :])
                        o2t.append(ok)
                acc6 = [ps6.tile([128, M], F32, name=f"a6_{p}", tag=f"a6_{p}")
                        for p in range(7)]
                for n in range(16):
                    for p in range(7):
                        nc.tensor.matmul(
                            acc6[p][:], wopt[n][:, p * 128:(p + 1) * 128],
                            o2t[n][:], start=(n == 0), stop=(n == 15))
                for p in range(7):
                    ob = s6o.tile([128, M], F16, name="outb", tag="outb")
                    nc.vector.tensor_copy(ob[:], acc6[p][:])
                    nc.sync.dma_start(outT[p * 128:(p + 1) * 128, :], ob[:])
            per_cm.__exit__(None, None, None)

    nc.compile()
    return nc


def prep_inputs(x, W_cqkv, W_uq, W_qk, kv_cache, W_o1, W_oproj, indices):
    x = np.asarray(x, np.float32)
    W_cqkv = np.asarray(W_cqkv, np.float32)
    W_uq = np.asarray(W_uq, np.float32)
    W_qk = np.asarray(W_qk, np.float32)
    kv_cache = np.asarray(kv_cache, np.float32)
    W_o1 = np.asarray(W_o1, np.float32)
    W_oproj = np.asarray(W_oproj, np.float32)
    indices = np.asarray(indices)

    xTf = np.ascontiguousarray(x.T).astype(NP16)
    W_eff = W_cqkv[:, 512:512 + 1536] @ W_uq          # [7168, 3072]
    kvT0 = kv_cache.T                                  # [576, 4096]
    kvTf = np.ascontiguousarray(
        np.concatenate([kvT0, kvT0[512:576]], axis=0)).astype(NP16)
    vvf = np.ascontiguousarray(kv_cache[:, :D_KV_C]).astype(NP16)
    cm = np.zeros((M, S_KV), np.float32)
    np.add.at(cm, (np.arange(M)[:, None], indices), 1.0)
    cntT = np.ascontiguousarray(cm.T).astype(NP16)

    in_maps = []
    for i in range(N_CORES):
        g0, g1 = 2 * i, 2 * i + 1
        wef = np.concatenate([
            W_eff[:, g0 * 192:g0 * 192 + 128],
            W_eff[:, g1 * 192:g1 * 192 + 128],
            W_eff[:, g0 * 192 + 128:(g0 + 1) * 192],
            W_eff[:, g1 * 192 + 128:(g1 + 1) * 192],
        ], axis=1).astype(NP16)
        # wop rows in gathered order: head-type major, then rank
        wop_rows = []
        for h in range(H_LOC):
            for rank in range(8):
                g = rank * H_LOC + h
                wop_rows.append(W_oproj[g * 128:(g + 1) * 128,
                                        i * OUT_C:(i + 1) * OUT_C])
        in_maps.append({
            "xT": xTf,
            "wef": np.ascontiguousarray(wef),
            "wqk": W_qk[g0:g1 + 1].astype(NP16),
            "kvT": kvTf,
            "vv": vvf,
            "cnt": cntT,
            "wo1": W_o1[g0:g1 + 1].astype(NP16),
            "wop": np.ascontiguousarray(np.concatenate(wop_rows, axis=0)
                                        ).astype(NP16),
        })
    return in_maps


_prog_cache = {}


def kernel(x, W_cqkv, W_uq, W_qk, kv_cache, W_o1, W_oproj, indices):
    if "nc" not in _prog_cache:
        _prog_cache["nc"] = build_program()
    nc = _prog_cache["nc"]
    in_maps = prep_inputs(x, W_cqkv, W_uq, W_qk, kv_cache, W_o1, W_oproj,
                          indices)
    trace = bool(int(os.environ.get("KERNEL_TRACE", "0")))
    res = run_bass_kernel_spmd(nc, in_maps, list(range(N_CORES)),
                               trace=trace)
    _prog_cache["last_result"] = res
    out = np.empty((M, HID), np.float32)
    for i in range(N_CORES):
        out[:, i * OUT_C:(i + 1) * OUT_C] = res.results[i]["outT"].T
    return out
